# revision 7
# baseline (speedup 1.0000x reference)
"""Distributed Trainium2 kernel for in-batch-negative InfoNCE loss.

loss = mean_i( logsumexp_j( cos(q_i, p_j)/T ) - cos(q_i, p_i)/T )

Strategy (8 NeuronCores, data-parallel over N):
  - each core owns a 1024-row shard of q and p (N=8192, D=768)
  - normalize local p shard, transpose to [D, 1024] bf16, AllGather it
    (two column-chunked AllGathers so compute can start earlier)
  - each core computes its 1024 x 8192 slab of logits with bf16 matmuls,
    fusing exp + row-sum into the ScalarE epilogue (logits never hit HBM)
  - q is NOT normalized before the matmul: the 1/(T*||q_i||) row scale is
    folded into the exp activation's per-partition scale
  - diagonal terms computed separately as elementwise dot of the local
    q/p shards (fp32), scaled by 1/(T*||q||*||p||)
  - per-core partial sum -> cross-partition ones-matmul -> scalar
    AllReduce -> loss
"""

import numpy as np

P = 128          # SBUF partitions
D = 768          # embedding dim
KT = D // P      # 6 k-tiles
NL = 1024        # local rows per core
MT = NL // P     # 8 m-tiles per core
NCORES = 8
N = NL * NCORES  # 8192
HALF = 512       # matmul moving free dim / half a rank block

_CACHE = {}


def _build(inv_temp: float, n_waves: int = NCORES):
    from concourse import bass, bacc, tile, mybir, masks

    f32 = mybir.dt.float32
    bf16 = mybir.dt.bfloat16
    AF = mybir.ActivationFunctionType
    ALU = mybir.AluOpType

    nc = bacc.Bacc(
        "TRN2",
        debug=False,
        target_bir_lowering=False,
        num_devices=NCORES,
    )

    q_ext = nc.dram_tensor("q", [NL, D], f32, kind="ExternalInput")
    p_ext = nc.dram_tensor("p", [NL, D], f32, kind="ExternalInput")
    out_ext = nc.dram_tensor("out", [1, 1], f32, kind="ExternalOutput")

    with tile.TileContext(nc) as tc:
        with (
            tc.tile_pool(name="singles", bufs=1) as singles,
            tc.tile_pool(name="persist", bufs=1) as persist,
            tc.tile_pool(name="pload", bufs=3) as pload,
            tc.tile_pool(name="qload", bufs=3) as qload,
            tc.tile_pool(name="pnorm", bufs=3) as pnorm,
            tc.tile_pool(name="scr", bufs=4) as scr,
            tc.tile_pool(name="escr", bufs=3) as escr,
            tc.tile_pool(name="stats", bufs=1) as stats,
            tc.tile_pool(name="dram", bufs=1, space="DRAM") as dram,
            tc.tile_pool(name="ps_tr", bufs=2, space="PSUM") as ps_tr,
            tc.tile_pool(name="ps_trb", bufs=2, space="PSUM") as ps_trb,
            tc.tile_pool(name="ps_mm", bufs=3, space="PSUM") as ps_mm,
            tc.tile_pool(name="ps_s", bufs=1, space="PSUM") as ps_s,
        ):
            ident_f = singles.tile([P, P], f32)
            masks.make_identity(nc, ident_f[:])
            ident_b = singles.tile([P, P], bf16)
            masks.make_identity(nc, ident_b[:])
            ones = singles.tile([P, 8], f32)
            nc.vector.memset(ones[:], 1.0)
            warm = singles.tile([P, 1], f32)
            nc.vector.memset(warm[:], 0.0)

            # persistent SBUF tensors
            qT = persist.tile([P, KT, NL], bf16)         # q^T (raw, bf16)
            pT_loc = persist.tile([P, KT, NL], bf16)     # local pn^T
            pT_a = persist.tile([P, NCORES * KT, HALF], bf16)  # gathered, cols 0:512
            pT_b = persist.tile([P, NCORES * KT, HALF], bf16)  # gathered, cols 512:1024

            # stats
            ssq_p = stats.tile([P, MT], f32)
            ssq_q = stats.tile([P, MT], f32)
            nrm = stats.tile([P, 2 * MT], f32)
            rp = stats.tile([P, MT], f32)       # 1/||p||
            rscale = stats.tile([P, MT], f32)   # 1/(T*||q||)
            draw = stats.tile([P, MT], f32)     # raw q.p dots
            dscaled = stats.tile([P, MT], f32)  # diag logits
            rs = stats.tile([P, MT * 16], f32)  # per-(mtile,ntile) exp row sums
            lse_pre = stats.tile([P, MT], f32)
            lse = stats.tile([P, MT], f32)
            vrow = stats.tile([P, MT], f32)     # lse - diag
            v1 = stats.tile([P, 1], f32)

            # DRAM bounce buffers for collectives
            ag_in_a = dram.tile([D, HALF], bf16)
            ag_in_b = dram.tile([D, HALF], bf16)
            ag_out_a = dram.tile([NCORES * D, HALF], bf16, addr_space="Shared")
            ag_out_b = dram.tile([NCORES * D, HALF], bf16, addr_space="Shared")
            ar_in = dram.tile([1, 8], f32)
            ar_out = dram.tile([1, 8], f32, addr_space="Shared")

            # warm the exp table early (avoids a 2.7us stall at first wave)
            nc.scalar.activation(warm[:], warm[:], AF.Exp)

            # ---------------- p path (critical: feeds the AllGather) ------
            # process tiles t=0..3 first (cols 0:512 of pn^T), launch AG a,
            # then t=4..7 and AG b.
            for t in range(MT):
                p_t = pload.tile([P, D], f32)
                nc.sync.dma_start(p_t[:], p_ext.ap()[t * P:(t + 1) * P, :])

                sq = scr.tile([P, D], f32)
                nc.scalar.activation(
                    sq[:], p_t[:], AF.Square, accum_out=ssq_p[:, t:t + 1]
                )
                nc.scalar.activation(
                    nrm[:, t:t + 1], ssq_p[:, t:t + 1], AF.Sqrt
                )
                nc.vector.reciprocal(rp[:, t:t + 1], nrm[:, t:t + 1])

                pn_t = pnorm.tile([P, D], bf16)
                nc.vector.tensor_scalar_mul(pn_t[:], p_t[:], rp[:, t:t + 1])

                # transpose 6 [128,128] blocks of pn_t into pT_loc
                for k in range(KT):
                    tp = ps_trb.tile([P, P], bf16)
                    nc.tensor.transpose(
                        tp[:], pn_t[:, k * P:(k + 1) * P], ident_b[:]
                    )
                    nc.vector.tensor_copy(
                        pT_loc[:, k, t * P:(t + 1) * P], tp[:]
                    )

                # q path interleaved (keeps DVE/ACT busy, not on AG path)
                q_t = qload.tile([P, D], f32)
                nc.sync.dma_start(q_t[:], q_ext.ap()[t * P:(t + 1) * P, :])

                dsc = scr.tile([P, D], f32)
                nc.vector.tensor_mul(dsc[:], q_t[:], p_t[:])
                nc.vector.tensor_reduce(
                    draw[:, t:t + 1], dsc[:], mybir.AxisListType.X, ALU.add
                )
                sq2 = scr.tile([P, D], f32)
                nc.scalar.activation(
                    sq2[:], q_t[:], AF.Square, accum_out=ssq_q[:, t:t + 1]
                )
                nc.scalar.activation(
                    nrm[:, MT + t:MT + t + 1], ssq_q[:, t:t + 1], AF.Sqrt
                )
                nc.vector.reciprocal(
                    rscale[:, t:t + 1], nrm[:, MT + t:MT + t + 1]
                )
                nc.vector.tensor_scalar_mul(
                    rscale[:, t:t + 1], rscale[:, t:t + 1], float(inv_temp)
                )
                # diag = (q.p) * rscale * rp
                nc.vector.tensor_mul(
                    dscaled[:, t:t + 1], draw[:, t:t + 1], rscale[:, t:t + 1]
                )
                nc.vector.tensor_mul(
                    dscaled[:, t:t + 1], dscaled[:, t:t + 1], rp[:, t:t + 1]
                )

                for k in range(KT):
                    tq = ps_tr.tile([P, P], f32)
                    nc.tensor.transpose(
                        tq[:], q_t[:, k * P:(k + 1) * P], ident_f[:]
                    )
                    nc.vector.tensor_copy(qT[:, k, t * P:(t + 1) * P], tq[:])

                # launch the chunked AllGathers as soon as their half of
                # pT_loc is complete
                if t == MT // 2 - 1:
                    nc.sync.dma_start(
                        ag_in_a[:].rearrange("(k p) m -> p k m", p=P),
                        pT_loc[:, :, 0:HALF],
                    )
                    nc.gpsimd.collective_compute(
                        "AllGather",
                        ALU.bypass,
                        replica_groups=[list(range(NCORES))],
                        ins=[ag_in_a[:].opt()],
                        outs=[ag_out_a[:].opt()],
                    )
                if t == MT - 1:
                    nc.sync.dma_start(
                        ag_in_b[:].rearrange("(k p) m -> p k m", p=P),
                        pT_loc[:, :, HALF:NL],
                    )
                    nc.gpsimd.collective_compute(
                        "AllGather",
                        ALU.bypass,
                        replica_groups=[list(range(NCORES))],
                        ins=[ag_in_b[:].opt()],
                        outs=[ag_out_b[:].opt()],
                    )

            # ------------- stream gathered blocks into SBUF ---------------
            for r in range(NCORES):
                nc.sync.dma_start(
                    pT_a[:, r * KT:(r + 1) * KT, :],
                    ag_out_a[:][r * D:(r + 1) * D, :].rearrange(
                        "(k p) m -> p k m", p=P
                    ),
                )
            for r in range(NCORES):
                nc.sync.dma_start(
                    pT_b[:, r * KT:(r + 1) * KT, :],
                    ag_out_b[:][r * D:(r + 1) * D, :].rearrange(
                        "(k p) m -> p k m", p=P
                    ),
                )

            # ---------------- main matmul waves ---------------------------
            # wave (r, h): columns of p-block r, half h; for each q m-tile,
            # 6 accumulating matmuls then fused exp+rowsum epilogue.
            nc.vector.memset(rs[:], 1.0)
            for r in range(n_waves):
                for h, pT_half in ((0, pT_a), (1, pT_b)):
                    for t in range(MT):
                        pm = ps_mm.tile([P, HALF], f32)
                        for k in range(KT):
                            nc.tensor.matmul(
                                pm[:],
                                qT[:, k, t * P:(t + 1) * P],
                                pT_half[:, r * KT + k, :],
                                start=(k == 0),
                                stop=(k == KT - 1),
                            )
                        ex = escr.tile([P, HALF], bf16)
                        nc.scalar.activation(
                            ex[:], pm[:], AF.Exp,
                            scale=rscale[:, t:t + 1],
                            accum_out=rs[:, (t * 16 + r * 2 + h):
                                          (t * 16 + r * 2 + h) + 1],
                        )

            # ---------------- epilogue ------------------------------------
            for t in range(MT):
                nc.vector.tensor_reduce(
                    lse_pre[:, t:t + 1],
                    rs[:, t * 16:(t + 1) * 16],
                    mybir.AxisListType.X,
                    ALU.add,
                )
            nc.scalar.activation(lse[:, :], lse_pre[:, :], AF.Ln)
            nc.vector.tensor_sub(vrow[:, :], lse[:, :], dscaled[:, :])
            nc.vector.tensor_reduce(
                v1[:], vrow[:, :], mybir.AxisListType.X, ALU.add
            )
            pssum = ps_s.tile([1, 8], f32)
            nc.tensor.matmul(pssum[:], v1[:], ones[:])
            ar_sb = stats.tile([1, 8], f32)
            nc.vector.tensor_scalar_mul(ar_sb[:], pssum[:], 1.0 / N)
            nc.sync.dma_start(ar_in[:], ar_sb[:])
            nc.gpsimd.collective_compute(
                "AllReduce",
                ALU.add,
                replica_groups=[list(range(NCORES))],
                ins=[ar_in[:].opt()],
                outs=[ar_out[:].opt()],
            )
            nc.sync.dma_start(out_ext.ap(), ar_out[:][0:1, 0:1])

    nc.compile()
    return nc


def _get_nc(inv_temp: float):
    import os
    n_waves = int(os.environ.get("NCE_N_WAVES", NCORES))
    key = (round(float(inv_temp), 9), n_waves)
    if key not in _CACHE:
        _CACHE[key] = _build(inv_temp, n_waves)
    return _CACHE[key]


def kernel(q_emb, p_emb, temperature):
    from concourse.bass_utils import run_bass_kernel_spmd

    q = np.ascontiguousarray(np.asarray(q_emb, dtype=np.float32))
    p = np.ascontiguousarray(np.asarray(p_emb, dtype=np.float32))
    t = float(np.asarray(temperature))
    nc = _get_nc(1.0 / t)

    in_maps = [
        {
            "q": q[i * NL:(i + 1) * NL],
            "p": p[i * NL:(i + 1) * NL],
        }
        for i in range(NCORES)
    ]
    res = run_bass_kernel_spmd(nc, in_maps, core_ids=list(range(NCORES)))
    return np.float32(res.results[0]["out"][0, 0])


# revision 13
# speedup vs baseline: 1.1391x; 1.1391x over previous
"""Distributed Trainium2 kernel for in-batch-negative InfoNCE loss.

loss = mean_i( logsumexp_j( cos(q_i, p_j)/T ) - cos(q_i, p_i)/T )

Strategy (8 NeuronCores, data-parallel over N):
  - each core owns a 1024-row shard of q and p (N=8192, D=768)
  - normalize local p shard, transpose to [D, 1024] bf16, AllGather it in
    two column-chunks (the first collective of a NEFF pays a ~70us entry
    barrier; chunking lets wave compute start after chunk A)
  - each core computes its 1024 x 8192 slab of logits with bf16 matmuls,
    fusing exp + row-sum into a single 1024-wide ScalarE epilogue per
    psum pair (logits never touch HBM)
  - q is NOT normalized before the matmul: the 1/(T*||q_i||) row scale is
    folded into the exp activation's per-partition scale
  - diagonal terms computed separately as an elementwise dot of the local
    q/pn shards, scaled by 1/(T*||q||)
  - a warm-up wave over the local block runs while the AllGather is in
    flight (its sums are discarded; it keeps the PE busy/warm)
  - per-core partial sums come back per core; the host adds the 8 scalars
"""

import numpy as np

P = 128          # SBUF partitions
D = 768          # embedding dim
KT = D // P      # 6 k-tiles
NL = 1024        # local rows per core
MT = NL // P     # 8 m-tiles per core
NCORES = 8
N = NL * NCORES  # 8192
HALF = 512       # column half per rank block
NSLOT = 10       # rs slots per m-tile: 8 real (h*4+rp) + 2 local warmup

_CACHE = {}


def _build(inv_temp: float, n_waves: int = NCORES):
    from concourse import bass, bacc, tile, mybir, masks

    f32 = mybir.dt.float32
    bf16 = mybir.dt.bfloat16
    AF = mybir.ActivationFunctionType
    ALU = mybir.AluOpType

    nc = bacc.Bacc(
        "TRN2",
        debug=False,
        target_bir_lowering=False,
        num_devices=NCORES,
    )

    q_ext = nc.dram_tensor("q", [NL, D], f32, kind="ExternalInput")
    p_ext = nc.dram_tensor("p", [NL, D], f32, kind="ExternalInput")
    out_ext = nc.dram_tensor("out", [1, 1], f32, kind="ExternalOutput")

    with tile.TileContext(nc) as tc:
        with (
            tc.tile_pool(name="singles", bufs=1) as singles,
            tc.tile_pool(name="persist", bufs=1) as persist,
            tc.tile_pool(name="pload", bufs=3) as pload,
            tc.tile_pool(name="qload", bufs=3) as qload,
            tc.tile_pool(name="scr", bufs=4) as scr,
            tc.tile_pool(name="escr", bufs=3) as escr,
            tc.tile_pool(name="stats", bufs=1) as stats,
            tc.tile_pool(name="dram", bufs=1, space="DRAM") as dram,
            tc.tile_pool(name="ps", bufs=4, space="PSUM") as ps,
        ):
            ident_f = singles.tile([P, P], f32)
            masks.make_identity(nc, ident_f[:])
            ident_b = singles.tile([P, P], bf16)
            masks.make_identity(nc, ident_b[:])
            ones = singles.tile([P, 8], f32)
            nc.vector.memset(ones[:], 1.0)
            warm = singles.tile([P, 1], f32)
            nc.vector.memset(warm[:], 0.0)

            # persistent SBUF tensors
            qT = persist.tile([P, KT, NL], bf16)          # q^T (raw, bf16)
            pT_loc = persist.tile([P, KT, NL], bf16)      # local pn^T
            pn_sb = persist.tile([P, MT, D], bf16)        # local pn (natural)
            pT_a = persist.tile([P, NCORES * KT, HALF], bf16)  # cols 0:512
            pT_b = persist.tile([P, NCORES * KT, HALF], bf16)  # cols 512:1024

            # stats
            ssq_p = stats.tile([P, MT], f32)
            ssq_q = stats.tile([P, MT], f32)
            nrm = stats.tile([P, 2 * MT], f32)
            rp = stats.tile([P, MT], f32)       # 1/||p||
            rscale = stats.tile([P, MT], f32)   # 1/(T*||q||)
            draw = stats.tile([P, MT], f32)     # raw q.pn dots
            dscaled = stats.tile([P, MT], f32)  # diag logits
            rs = stats.tile([P, MT * NSLOT], f32)
            lse_pre = stats.tile([P, MT], f32)
            lse = stats.tile([P, MT], f32)
            vrow = stats.tile([P, MT], f32)
            v1 = stats.tile([P, 1], f32)
            ar_sb = stats.tile([1, 8], f32)

            # DRAM bounce buffers for the collectives
            ag_in_a = dram.tile([D, HALF], bf16)
            ag_in_b = dram.tile([D, HALF], bf16)
            ag_out_a = dram.tile([NCORES * D, HALF], bf16, addr_space="Shared")
            ag_out_b = dram.tile([NCORES * D, HALF], bf16, addr_space="Shared")

            # warm the exp table early
            nc.scalar.activation(warm[:], warm[:], AF.Exp)
            nc.vector.memset(rs[:], 1.0)

            def p_tile(t):
                p_t = pload.tile([P, D], f32, name="p_t")
                nc.sync.dma_start(p_t[:], p_ext.ap()[t * P:(t + 1) * P, :])
                sq = scr.tile([P, D], f32, name="sq")
                nc.scalar.activation(
                    sq[:], p_t[:], AF.Square, accum_out=ssq_p[:, t:t + 1]
                )
                nc.scalar.activation(nrm[:, t:t + 1], ssq_p[:, t:t + 1], AF.Sqrt)
                nc.vector.reciprocal(rp[:, t:t + 1], nrm[:, t:t + 1])
                nc.vector.tensor_scalar_mul(
                    pn_sb[:, t, :], p_t[:], rp[:, t:t + 1]
                )
                for k in range(KT):
                    tp = ps.tile([P, P], bf16, tag="ps", name="tp")
                    nc.tensor.transpose(
                        tp[:], pn_sb[:, t, k * P:(k + 1) * P], ident_b[:]
                    )
                    nc.vector.tensor_copy(pT_loc[:, k, t * P:(t + 1) * P], tp[:])

            def q_tile(t):
                q_t = qload.tile([P, D], f32, name="q_t")
                nc.sync.dma_start(q_t[:], q_ext.ap()[t * P:(t + 1) * P, :])
                sq2 = scr.tile([P, D], f32, name="sq2")
                nc.scalar.activation(
                    sq2[:], q_t[:], AF.Square, accum_out=ssq_q[:, t:t + 1]
                )
                nc.scalar.activation(
                    nrm[:, MT + t:MT + t + 1], ssq_q[:, t:t + 1], AF.Sqrt
                )
                nc.vector.reciprocal(
                    rscale[:, t:t + 1], nrm[:, MT + t:MT + t + 1]
                )
                nc.vector.tensor_scalar_mul(
                    rscale[:, t:t + 1], rscale[:, t:t + 1], float(inv_temp)
                )
                # diag = (q . pn) * rscale
                dsc = scr.tile([P, D], f32, name="dsc")
                nc.vector.tensor_mul(dsc[:], q_t[:], pn_sb[:, t, :])
                nc.vector.tensor_reduce(
                    draw[:, t:t + 1], dsc[:], mybir.AxisListType.X, ALU.add
                )
                nc.vector.tensor_mul(
                    dscaled[:, t:t + 1], draw[:, t:t + 1], rscale[:, t:t + 1]
                )
                for k in range(KT):
                    tq = ps.tile([P, P], f32, tag="ps", name="tq")
                    nc.tensor.transpose(
                        tq[:], q_t[:, k * P:(k + 1) * P], ident_f[:]
                    )
                    nc.vector.tensor_copy(qT[:, k, t * P:(t + 1) * P], tq[:])

            def wave_pair(t, rhs_left, rhs_right, slot):
                """12 accumulating matmuls into a [128,1024] psum pair, then
                one fused exp+rowsum. rhs_{left,right} map k -> [128,512] AP."""
                pm = ps.tile([P, 2 * HALF], f32, tag="ps", name="pm")
                for k in range(KT):
                    lhsT = qT[:, k, t * P:(t + 1) * P]
                    nc.tensor.matmul(
                        pm[:, 0:HALF], lhsT, rhs_left(k),
                        start=(k == 0), stop=(k == KT - 1),
                    )
                    nc.tensor.matmul(
                        pm[:, HALF:2 * HALF], lhsT, rhs_right(k),
                        start=(k == 0), stop=(k == KT - 1),
                    )
                ex = escr.tile([P, 2 * HALF], bf16, name="ex")
                nc.scalar.activation(
                    ex[:], pm[:], AF.Exp,
                    scale=rscale[:, t:t + 1],
                    accum_out=rs[:, slot:slot + 1],
                )

            # ---- p path (first half) -> AllGather A ----------------------
            for t in range(MT // 2):
                p_tile(t)
            nc.sync.dma_start(
                ag_in_a[:].rearrange("(k p) m -> p k m", p=P),
                pT_loc[:, :, 0:HALF],
            )
            nc.gpsimd.collective_compute(
                "AllGather", ALU.bypass,
                replica_groups=[list(range(NCORES))],
                ins=[ag_in_a[:].opt()], outs=[ag_out_a[:].opt()],
            )
            # ---- p path (second half) ------------------------------------
            for t in range(MT // 2, MT):
                p_tile(t)

            # ---- q path --------------------------------------------------
            for t in range(MT):
                q_tile(t)

            # ---- AllGather B ---------------------------------------------
            nc.sync.dma_start(
                ag_in_b[:].rearrange("(k p) m -> p k m", p=P),
                pT_loc[:, :, HALF:NL],
            )
            nc.gpsimd.collective_compute(
                "AllGather", ALU.bypass,
                replica_groups=[list(range(NCORES))],
                ins=[ag_in_b[:].opt()], outs=[ag_out_b[:].opt()],
            )

            # ---- local warm-up wave (results discarded) ------------------
            for t in range(MT):
                wave_pair(
                    t,
                    lambda k: pT_loc[:, k, 0:HALF],
                    lambda k: pT_loc[:, k, HALF:2 * HALF],
                    t * NSLOT + 8,
                )

            # ---- stream gathered blocks into SBUF (gpsimd: idle engine,
            # keeps the long AG waits off the compute streams) -------------
            for r in range(NCORES):
                nc.gpsimd.dma_start(
                    pT_a[:, r * KT:(r + 1) * KT, :],
                    ag_out_a[:][r * D:(r + 1) * D, :].rearrange(
                        "(k p) m -> p k m", p=P
                    ),
                )
            for r in range(NCORES):
                nc.gpsimd.dma_start(
                    pT_b[:, r * KT:(r + 1) * KT, :],
                    ag_out_b[:][r * D:(r + 1) * D, :].rearrange(
                        "(k p) m -> p k m", p=P
                    ),
                )

            # ---- main waves: h outer, rank-pairs inner -------------------
            for h, pT_h in ((0, pT_a), (1, pT_b)):
                for rpair in range(n_waves // 2):
                    r0, r1 = 2 * rpair, 2 * rpair + 1
                    for t in range(MT):
                        wave_pair(
                            t,
                            lambda k, r0=r0, pT_h=pT_h: pT_h[:, r0 * KT + k, :],
                            lambda k, r1=r1, pT_h=pT_h: pT_h[:, r1 * KT + k, :],
                            t * NSLOT + h * 4 + rpair,
                        )

            # ---- epilogue ------------------------------------------------
            for t in range(MT):
                nc.vector.tensor_reduce(
                    lse_pre[:, t:t + 1],
                    rs[:, t * NSLOT:t * NSLOT + 8],
                    mybir.AxisListType.X,
                    ALU.add,
                )
            nc.scalar.activation(lse[:, :], lse_pre[:, :], AF.Ln)
            nc.vector.tensor_sub(vrow[:, :], lse[:, :], dscaled[:, :])
            nc.vector.tensor_reduce(
                v1[:], vrow[:, :], mybir.AxisListType.X, ALU.add
            )
            pssum = ps.tile([1, 8], f32, tag="ps", name="pssum")
            nc.tensor.matmul(pssum[:], v1[:], ones[:])
            nc.vector.tensor_scalar_mul(ar_sb[:], pssum[:], 1.0 / N)
            nc.sync.dma_start(out_ext.ap(), ar_sb[:, 0:1])

    nc.compile()
    return nc


def _get_nc(inv_temp: float):
    import os
    n_waves = int(os.environ.get("NCE_N_WAVES", NCORES))
    key = (round(float(inv_temp), 9), n_waves)
    if key not in _CACHE:
        _CACHE[key] = _build(inv_temp, n_waves)
    return _CACHE[key]


def kernel(q_emb, p_emb, temperature):
    from concourse.bass_utils import run_bass_kernel_spmd

    q = np.ascontiguousarray(np.asarray(q_emb, dtype=np.float32))
    p = np.ascontiguousarray(np.asarray(p_emb, dtype=np.float32))
    t = float(np.asarray(temperature))
    nc = _get_nc(1.0 / t)

    in_maps = [
        {
            "q": q[i * NL:(i + 1) * NL],
            "p": p[i * NL:(i + 1) * NL],
        }
        for i in range(NCORES)
    ]
    res = run_bass_kernel_spmd(nc, in_maps, core_ids=list(range(NCORES)))
    return np.float32(sum(float(r["out"][0, 0]) for r in res.results))


# revision 14
# speedup vs baseline: 1.2953x; 1.1371x over previous
"""Distributed Trainium2 kernel for in-batch-negative InfoNCE loss.

loss = mean_i( logsumexp_j( cos(q_i, p_j)/T ) - cos(q_i, p_i)/T )

Strategy (8 NeuronCores, data-parallel over N):
  - each core owns a 1024-row shard of q and p (N=8192, D=768)
  - normalize local p shard (x16), transpose into the fp8 DoubleRow
    interleaved layout [d/256, 2, m], AllGather it in two column-chunks
    (the first collective of a NEFF pays a ~60-70us entry barrier;
    chunking lets h=0 wave compute overlap the second chunk)
  - each core computes its 1024 x 8192 slab of logits with fp8e4m3
    DoubleRow matmuls (157 TF/s), fusing exp + row-sum into a 1024-wide
    ScalarE epilogue per psum pair (logits never touch HBM)
  - q is scaled x8 and NOT normalized before the matmul: the
    1/(128*T*||q_i||) row scale is folded into the exp activation
  - diagonal terms computed separately as an elementwise dot of the
    local q/pn shards in f32/bf16 precision
  - a warm-up wave over the local block runs while the AllGather is in
    flight (sums discarded; keeps PE busy and the HAM clock warm)
  - per-core partial sums are returned per core; host adds 8 scalars
"""

import numpy as np

P = 128          # SBUF partitions
D = 768          # embedding dim
K2 = 3           # 256-deep DoubleRow k-tiles
NL = 1024        # local rows per core
MT = NL // P     # 8 m-tiles per core
NCORES = 8
N = NL * NCORES  # 8192
HALF = 512       # column half per rank block
NSLOT = 10       # rs slots per m-tile: 8 real (h*4+rp) + 1 local + pad
QS = 8.0         # q fp8 pre-scale
PS = 16.0        # pn fp8 pre-scale

_CACHE = {}


def _build(inv_temp: float, n_waves: int = NCORES):
    from concourse import bass, bacc, tile, mybir, masks

    f32 = mybir.dt.float32
    bf16 = mybir.dt.bfloat16
    fp8 = mybir.dt.float8e4
    AF = mybir.ActivationFunctionType
    ALU = mybir.AluOpType
    DR = mybir.MatmulPerfMode.DoubleRow

    nc = bacc.Bacc(
        "TRN2",
        debug=False,
        target_bir_lowering=False,
        num_devices=NCORES,
    )

    q_ext = nc.dram_tensor("q", [NL, D], f32, kind="ExternalInput")
    p_ext = nc.dram_tensor("p", [NL, D], f32, kind="ExternalInput")
    out_ext = nc.dram_tensor("out", [1, 1], f32, kind="ExternalOutput")

    with tile.TileContext(nc) as tc:
        with (
            tc.tile_pool(name="singles", bufs=1) as singles,
            tc.tile_pool(name="persist", bufs=1) as persist,
            tc.tile_pool(name="pload", bufs=3) as pload,
            tc.tile_pool(name="qload", bufs=3) as qload,
            tc.tile_pool(name="scr", bufs=4) as scr,
            tc.tile_pool(name="escr", bufs=3) as escr,
            tc.tile_pool(name="stats", bufs=1) as stats,
            tc.tile_pool(name="dram", bufs=1, space="DRAM") as dram,
            tc.tile_pool(name="ps", bufs=3, space="PSUM") as ps,
            tc.tile_pool(name="pstr", bufs=2, space="PSUM") as pstr,
        ):
            ident_f = singles.tile([P, P], f32)
            masks.make_identity(nc, ident_f[:])
            ident_b = singles.tile([P, P], bf16)
            masks.make_identity(nc, ident_b[:])
            ones = singles.tile([P, 8], f32)
            nc.vector.memset(ones[:], 1.0)
            warm = singles.tile([P, 1], f32)
            nc.vector.memset(warm[:], 0.0)

            # persistent SBUF tensors (fp8 DoubleRow interleaved layouts)
            qT8 = persist.tile([P, K2, 2, NL], fp8)       # 8*q^T
            pT_loc = persist.tile([P, K2, 2, NL], fp8)    # local 16*pn^T
            pn_sb = persist.tile([P, MT, D], bf16)        # local 16*pn
            pT_a = persist.tile([P, NCORES * K2, 2, HALF], fp8)  # cols 0:512
            pT_b = persist.tile([P, NCORES * K2, 2, HALF], fp8)  # cols 512:1024

            # stats
            ssq_p = stats.tile([P, MT], f32)
            ssq_q = stats.tile([P, MT], f32)
            nrm = stats.tile([P, 2 * MT], f32)
            rp = stats.tile([P, MT], f32)        # 16/||p||
            rscale = stats.tile([P, MT], f32)    # 1/(128*T*||q||)
            draw = stats.tile([P, MT], f32)      # raw q.pn16 dots
            dscaled = stats.tile([P, MT], f32)   # diag logits
            rs = stats.tile([P, MT * NSLOT], f32)
            lse_pre = stats.tile([P, MT], f32)
            lse = stats.tile([P, MT], f32)
            vrow = stats.tile([P, MT], f32)
            v1 = stats.tile([P, 1], f32)
            ar_sb = stats.tile([1, 8], f32)

            # DRAM bounce buffers for the collectives
            ag_in_a = dram.tile([D, HALF], fp8)
            ag_in_b = dram.tile([D, HALF], fp8)
            ag_out_a = dram.tile([NCORES * D, HALF], fp8, addr_space="Shared")
            ag_out_b = dram.tile([NCORES * D, HALF], fp8, addr_space="Shared")

            # warm the exp table early
            nc.scalar.activation(warm[:], warm[:], AF.Exp)
            nc.vector.memset(rs[:], 1.0)

            def p_tile(t):
                p_t = pload.tile([P, D], f32, name="p_t")
                nc.sync.dma_start(p_t[:], p_ext.ap()[t * P:(t + 1) * P, :])
                sq = scr.tile([P, D], f32, name="sq")
                nc.scalar.activation(
                    sq[:], p_t[:], AF.Square, accum_out=ssq_p[:, t:t + 1]
                )
                nc.scalar.activation(nrm[:, t:t + 1], ssq_p[:, t:t + 1], AF.Sqrt)
                nc.vector.reciprocal(rp[:, t:t + 1], nrm[:, t:t + 1])
                nc.vector.tensor_scalar_mul(
                    rp[:, t:t + 1], rp[:, t:t + 1], float(PS)
                )
                nc.vector.tensor_scalar_mul(
                    pn_sb[:, t, :], p_t[:], rp[:, t:t + 1]
                )
                pv = pn_sb[:, t, :].rearrange(
                    "p (k2 c two) -> p k2 two c", k2=K2, two=2
                )
                for k2 in range(K2):
                    for s in range(2):
                        tp = pstr.tile([P, P], bf16, tag="pstr", name="tp")
                        nc.tensor.transpose(tp[:], pv[:, k2, s, :], ident_b[:])
                        nc.vector.tensor_copy(
                            pT_loc[:, k2, s, t * P:(t + 1) * P], tp[:]
                        )

            def q_tile(t):
                q_t = qload.tile([P, D], f32, name="q_t")
                nc.sync.dma_start(q_t[:], q_ext.ap()[t * P:(t + 1) * P, :])
                sq2 = scr.tile([P, D], f32, name="sq2")
                nc.scalar.activation(
                    sq2[:], q_t[:], AF.Square, accum_out=ssq_q[:, t:t + 1]
                )
                nc.scalar.activation(
                    nrm[:, MT + t:MT + t + 1], ssq_q[:, t:t + 1], AF.Sqrt
                )
                nc.vector.reciprocal(
                    rscale[:, t:t + 1], nrm[:, MT + t:MT + t + 1]
                )
                nc.vector.tensor_scalar_mul(
                    rscale[:, t:t + 1], rscale[:, t:t + 1],
                    float(inv_temp) / float(QS * PS),
                )
                # diag = (q . pn16) * rscale * QS   (rscale has 1/(QS*PS*T))
                dsc = scr.tile([P, D], f32, name="dsc")
                nc.vector.tensor_mul(dsc[:], q_t[:], pn_sb[:, t, :])
                nc.vector.tensor_reduce(
                    draw[:, t:t + 1], dsc[:], mybir.AxisListType.X, ALU.add
                )
                nc.vector.tensor_mul(
                    dscaled[:, t:t + 1], draw[:, t:t + 1], rscale[:, t:t + 1]
                )
                nc.vector.tensor_scalar_mul(
                    dscaled[:, t:t + 1], dscaled[:, t:t + 1], float(QS)
                )
                qv = q_t[:].rearrange(
                    "p (k2 c two) -> p k2 two c", k2=K2, two=2
                )
                for k2 in range(K2):
                    for s in range(2):
                        tq = pstr.tile([P, P], f32, tag="pstr", name="tq")
                        nc.tensor.transpose(tq[:], qv[:, k2, s, :], ident_f[:])
                        nc.vector.tensor_scalar_mul(
                            qT8[:, k2, s, t * P:(t + 1) * P], tq[:], float(QS)
                        )

            def wave_pair(t, rhs_left, rhs_right, slot):
                """6 DoubleRow matmuls into a [128,1024] psum pair, then one
                fused exp+rowsum. rhs_{left,right}: k2 -> [128,2,512] AP."""
                pm = ps.tile([P, 2 * HALF], f32, tag="ps", name="pm")
                for k2 in range(K2):
                    lhsT = qT8[:, k2, :, t * P:(t + 1) * P]
                    nc.tensor.matmul(
                        pm[:, 0:HALF], lhsT, rhs_left(k2),
                        start=(k2 == 0), stop=(k2 == K2 - 1), perf_mode=DR,
                    )
                    nc.tensor.matmul(
                        pm[:, HALF:2 * HALF], lhsT, rhs_right(k2),
                        start=(k2 == 0), stop=(k2 == K2 - 1), perf_mode=DR,
                    )
                ex = escr.tile([P, 2 * HALF], bf16, name="ex")
                nc.scalar.activation(
                    ex[:], pm[:], AF.Exp,
                    scale=rscale[:, t:t + 1],
                    accum_out=rs[:, slot:slot + 1],
                )

            # ---- p path (first half) -> AllGather A ----------------------
            for t in range(MT // 2):
                p_tile(t)
            nc.sync.dma_start(
                ag_in_a[:].rearrange("(k2 two p) m -> p k2 two m", p=P, two=2),
                pT_loc[:, :, :, 0:HALF],
            )
            nc.gpsimd.collective_compute(
                "AllGather", ALU.bypass,
                replica_groups=[list(range(NCORES))],
                ins=[ag_in_a[:].opt()], outs=[ag_out_a[:].opt()],
            )
            # ---- p path (second half) ------------------------------------
            for t in range(MT // 2, MT):
                p_tile(t)

            # ---- q path --------------------------------------------------
            for t in range(MT):
                q_tile(t)

            # ---- AllGather B ---------------------------------------------
            nc.sync.dma_start(
                ag_in_b[:].rearrange("(k2 two p) m -> p k2 two m", p=P, two=2),
                pT_loc[:, :, :, HALF:NL],
            )
            nc.gpsimd.collective_compute(
                "AllGather", ALU.bypass,
                replica_groups=[list(range(NCORES))],
                ins=[ag_in_b[:].opt()], outs=[ag_out_b[:].opt()],
            )

            # ---- local warm-up wave (results discarded) ------------------
            for t in range(MT):
                wave_pair(
                    t,
                    lambda k2: pT_loc[:, k2, :, 0:HALF],
                    lambda k2: pT_loc[:, k2, :, HALF:2 * HALF],
                    t * NSLOT + 8,
                )

            # ---- stream gathered blocks into SBUF (gpsimd: idle engine,
            # keeps the long AG waits off the compute streams) -------------
            for r in range(NCORES):
                nc.gpsimd.dma_start(
                    pT_a[:, r * K2:(r + 1) * K2, :, :],
                    ag_out_a[:][r * D:(r + 1) * D, :].rearrange(
                        "(k2 two p) m -> p k2 two m", p=P, two=2
                    ),
                )
            for r in range(NCORES):
                nc.gpsimd.dma_start(
                    pT_b[:, r * K2:(r + 1) * K2, :, :],
                    ag_out_b[:][r * D:(r + 1) * D, :].rearrange(
                        "(k2 two p) m -> p k2 two m", p=P, two=2
                    ),
                )

            # ---- main waves: h outer, rank-pairs inner -------------------
            for h, pT_h in ((0, pT_a), (1, pT_b)):
                for rpair in range(n_waves // 2):
                    r0, r1 = 2 * rpair, 2 * rpair + 1
                    for t in range(MT):
                        wave_pair(
                            t,
                            lambda k2, r0=r0, pT_h=pT_h:
                                pT_h[:, r0 * K2 + k2, :, :],
                            lambda k2, r1=r1, pT_h=pT_h:
                                pT_h[:, r1 * K2 + k2, :, :],
                            t * NSLOT + h * 4 + rpair,
                        )

            # ---- epilogue ------------------------------------------------
            for t in range(MT):
                nc.vector.tensor_reduce(
                    lse_pre[:, t:t + 1],
                    rs[:, t * NSLOT:t * NSLOT + 8],
                    mybir.AxisListType.X,
                    ALU.add,
                )
            nc.scalar.activation(lse[:, :], lse_pre[:, :], AF.Ln)
            nc.vector.tensor_sub(vrow[:, :], lse[:, :], dscaled[:, :])
            nc.vector.tensor_reduce(
                v1[:], vrow[:, :], mybir.AxisListType.X, ALU.add
            )
            pssum = ps.tile([1, 8], f32, tag="ps", name="pssum")
            nc.tensor.matmul(pssum[:], v1[:], ones[:])
            nc.vector.tensor_scalar_mul(ar_sb[:], pssum[:], 1.0 / N)
            nc.sync.dma_start(out_ext.ap(), ar_sb[:, 0:1])

    nc.compile()
    return nc


def _get_nc(inv_temp: float):
    import os
    n_waves = int(os.environ.get("NCE_N_WAVES", NCORES))
    key = (round(float(inv_temp), 9), n_waves)
    if key not in _CACHE:
        _CACHE[key] = _build(inv_temp, n_waves)
    return _CACHE[key]


def kernel(q_emb, p_emb, temperature):
    from concourse.bass_utils import run_bass_kernel_spmd

    q = np.ascontiguousarray(np.asarray(q_emb, dtype=np.float32))
    p = np.ascontiguousarray(np.asarray(p_emb, dtype=np.float32))
    t = float(np.asarray(temperature))
    nc = _get_nc(1.0 / t)

    in_maps = [
        {
            "q": q[i * NL:(i + 1) * NL],
            "p": p[i * NL:(i + 1) * NL],
        }
        for i in range(NCORES)
    ]
    res = run_bass_kernel_spmd(nc, in_maps, core_ids=list(range(NCORES)))
    return np.float32(sum(float(r["out"][0, 0]) for r in res.results))


# revision 19
# speedup vs baseline: 1.5943x; 1.2308x over previous
"""Distributed Trainium2 kernel for in-batch-negative InfoNCE loss.

loss = mean_i( logsumexp_j( cos(q_i, p_j)/T ) - cos(q_i, p_i)/T )

Strategy (8 NeuronCores, data-parallel over N):
  - each core owns a 1024-row shard of q and p (N=8192, D=768)
  - normalize local p shard (x16), transpose into the fp8 DoubleRow
    interleaved layout [d/256, 2, m], AllGather it in two column-chunks
    (the first collective of a NEFF pays a ~60-70us entry barrier;
    chunking lets h=0 wave compute overlap the second chunk)
  - each core computes its 1024 x 8192 slab of logits with fp8e4m3
    DoubleRow matmuls (157 TF/s), fusing exp + row-sum into a 1024-wide
    ScalarE epilogue per psum pair (logits never touch HBM)
  - q is scaled x8 and NOT normalized before the matmul: the
    1/(128*T*||q_i||) row scale is folded into the exp activation
  - diagonal terms computed separately as an elementwise dot of the
    local q/pn shards in f32/bf16 precision
  - a warm-up wave over the local block runs while the AllGather is in
    flight (sums discarded; keeps PE busy and the HAM clock warm)
  - per-core partial sums are returned per core; host adds 8 scalars
"""

import numpy as np

P = 128          # SBUF partitions
D = 768          # embedding dim
K2 = 3           # 256-deep DoubleRow k-tiles
NL = 1024        # local rows per core
MT = NL // P     # 8 m-tiles per core
NCORES = 8
N = NL * NCORES  # 8192
HALF = 512       # column half per rank block
NSLOT = 10       # rs slots per m-tile: 8 real (h*4+rp) + 1 local + pad
QS = 8.0         # q fp8 pre-scale
PS = 16.0        # pn fp8 pre-scale

_CACHE = {}


def _build(inv_temp: float, n_waves: int = NCORES):
    from concourse import bass, bacc, tile, mybir, masks

    f32 = mybir.dt.float32
    bf16 = mybir.dt.bfloat16
    fp8 = mybir.dt.float8e4
    AF = mybir.ActivationFunctionType
    ALU = mybir.AluOpType
    DR = mybir.MatmulPerfMode.DoubleRow

    nc = bacc.Bacc(
        "TRN2",
        debug=False,
        target_bir_lowering=False,
        num_devices=NCORES,
    )

    q_ext = nc.dram_tensor("q", [NL, D], f32, kind="ExternalInput")
    p_ext = nc.dram_tensor("p", [NL, D], f32, kind="ExternalInput")
    out_ext = nc.dram_tensor("out", [1, 1], f32, kind="ExternalOutput")

    with tile.TileContext(nc) as tc:
        with (
            tc.tile_pool(name="singles", bufs=1) as singles,
            tc.tile_pool(name="persist", bufs=1) as persist,
            tc.tile_pool(name="pload", bufs=3) as pload,
            tc.tile_pool(name="qload", bufs=3) as qload,
            tc.tile_pool(name="scr", bufs=4) as scr,
            tc.tile_pool(name="escr", bufs=3) as escr,
            tc.tile_pool(name="stats", bufs=1) as stats,
            tc.tile_pool(name="dram", bufs=1, space="DRAM") as dram,
            tc.tile_pool(name="ps", bufs=3, space="PSUM") as ps,
            tc.tile_pool(name="pstr", bufs=2, space="PSUM") as pstr,
        ):
            ident_f = singles.tile([P, P], f32)
            masks.make_identity(nc, ident_f[:])
            ident_b = singles.tile([P, P], bf16)
            masks.make_identity(nc, ident_b[:])
            ones = singles.tile([P, 8], f32)
            nc.vector.memset(ones[:], 1.0)
            warm = singles.tile([P, 1], f32)
            nc.vector.memset(warm[:], 0.0)
            bias_p = singles.tile([P, 1], f32)
            nc.vector.memset(bias_p[:], float(np.log(PS)))
            bias_q = singles.tile([P, 1], f32)
            nc.vector.memset(
                bias_q[:], float(np.log(float(inv_temp) / float(QS * PS)))
            )

            # persistent SBUF tensors (fp8 DoubleRow interleaved layouts)
            qT8 = persist.tile([P, K2, 2, NL], fp8)       # 8*q^T
            pT_loc = persist.tile([P, K2, 2, NL], fp8)    # local 16*pn^T
            pn_sb = persist.tile([P, MT, D], bf16)        # local 16*pn
            pT_a = persist.tile([P, NCORES * K2, 2, HALF], fp8)  # cols 0:512
            pT_b = persist.tile([P, NCORES * K2, 2, HALF], fp8)  # cols 512:1024

            # stats
            ssq_p = stats.tile([P, MT], f32)
            ssq_q = stats.tile([P, MT], f32)
            nrm = stats.tile([P, 2 * MT], f32)
            rp = stats.tile([P, MT], f32)        # 16/||p||
            rscale = stats.tile([P, MT], f32)    # 1/(128*T*||q||)
            draw = stats.tile([P, MT], f32)      # raw q.pn16 dots
            dscaled = stats.tile([P, MT], f32)   # diag logits
            rs = stats.tile([P, MT * NSLOT], f32)
            lse_pre = stats.tile([P, MT], f32)
            lse = stats.tile([P, MT], f32)
            vrow = stats.tile([P, MT], f32)
            v1 = stats.tile([P, 1], f32)
            ar_sb = stats.tile([1, 8], f32)

            # DRAM bounce buffers for the collectives
            ag_in_a = dram.tile([D, HALF], fp8)
            ag_in_b = dram.tile([D, HALF], fp8)
            ag_out_a = dram.tile([NCORES * D, HALF], fp8, addr_space="Shared")
            ag_out_b = dram.tile([NCORES * D, HALF], fp8, addr_space="Shared")

            # warm the exp table early
            nc.scalar.activation(warm[:], warm[:], AF.Exp)
            nc.vector.memset(rs[:], 1.0)

            def p_tile(t):
                p_t = pload.tile([P, D], f32, name="p_t")
                nc.sync.dma_start(p_t[:], p_ext.ap()[t * P:(t + 1) * P, :])
                sq = scr.tile([P, D], f32, name="sq")
                nc.scalar.activation(
                    sq[:], p_t[:], AF.Square, accum_out=ssq_p[:, t:t + 1]
                )
                # rp = PS/||p|| = exp(-0.5*ln(ssq) + ln(PS)) — stays in the
                # natural_log/exp table set (no Sqrt set swap)
                nc.scalar.activation(nrm[:, t:t + 1], ssq_p[:, t:t + 1], AF.Ln)
                nc.scalar.activation(
                    rp[:, t:t + 1], nrm[:, t:t + 1], AF.Exp,
                    scale=-0.5, bias=bias_p[:],
                )
                nc.vector.tensor_scalar_mul(
                    pn_sb[:, t, :], p_t[:], rp[:, t:t + 1]
                )
                pv = pn_sb[:, t, :].rearrange(
                    "p (k2 c two) -> p k2 two c", k2=K2, two=2
                )
                for k2 in range(K2):
                    for s in range(2):
                        tp = pstr.tile([P, P], bf16, tag="pstr", name="tp")
                        nc.tensor.transpose(tp[:], pv[:, k2, s, :], ident_b[:])
                        nc.vector.tensor_copy(
                            pT_loc[:, k2, s, t * P:(t + 1) * P], tp[:]
                        )

            def q_tile(t):
                q_t = qload.tile([P, D], f32, name="q_t")
                nc.sync.dma_start(q_t[:], q_ext.ap()[t * P:(t + 1) * P, :])
                sq2 = scr.tile([P, D], f32, name="sq2")
                nc.scalar.activation(
                    sq2[:], q_t[:], AF.Square, accum_out=ssq_q[:, t:t + 1]
                )
                # rscale = inv_temp/(QS*PS*||q||) via the same ln/exp set
                nc.scalar.activation(
                    nrm[:, MT + t:MT + t + 1], ssq_q[:, t:t + 1], AF.Ln
                )
                nc.scalar.activation(
                    rscale[:, t:t + 1], nrm[:, MT + t:MT + t + 1], AF.Exp,
                    scale=-0.5, bias=bias_q[:],
                )
                # diag = (q . pn16) * rscale * QS   (rscale has 1/(QS*PS*T))
                dsc = scr.tile([P, D], f32, name="dsc")
                nc.vector.tensor_mul(dsc[:], q_t[:], pn_sb[:, t, :])
                nc.vector.tensor_reduce(
                    draw[:, t:t + 1], dsc[:], mybir.AxisListType.X, ALU.add
                )
                nc.vector.tensor_mul(
                    dscaled[:, t:t + 1], draw[:, t:t + 1], rscale[:, t:t + 1]
                )
                nc.vector.tensor_scalar_mul(
                    dscaled[:, t:t + 1], dscaled[:, t:t + 1], float(QS)
                )
                qv = q_t[:].rearrange(
                    "p (k2 c two) -> p k2 two c", k2=K2, two=2
                )
                for k2 in range(K2):
                    for s in range(2):
                        tq = pstr.tile([P, P], f32, tag="pstr", name="tq")
                        nc.tensor.transpose(tq[:], qv[:, k2, s, :], ident_f[:])
                        nc.vector.tensor_scalar_mul(
                            qT8[:, k2, s, t * P:(t + 1) * P], tq[:], float(QS)
                        )

            def wave_pair(t, rhs_left, rhs_right, slot):
                """6 DoubleRow matmuls into a [128,1024] psum pair, then one
                fused exp+rowsum. rhs_{left,right}: k2 -> [128,2,512] AP."""
                pm = ps.tile([P, 2 * HALF], f32, tag="ps", name="pm")
                for k2 in range(K2):
                    lhsT = qT8[:, k2, :, t * P:(t + 1) * P]
                    nc.tensor.matmul(
                        pm[:, 0:HALF], lhsT, rhs_left(k2),
                        start=(k2 == 0), stop=(k2 == K2 - 1), perf_mode=DR,
                    )
                    nc.tensor.matmul(
                        pm[:, HALF:2 * HALF], lhsT, rhs_right(k2),
                        start=(k2 == 0), stop=(k2 == K2 - 1), perf_mode=DR,
                    )
                ex = escr.tile([P, 2 * HALF], bf16, name="ex")
                nc.scalar.activation(
                    ex[:], pm[:], AF.Exp,
                    scale=rscale[:, t:t + 1],
                    accum_out=rs[:, slot:slot + 1],
                )

            # ---- p path (first half) -> AllGather A ----------------------
            for t in range(MT // 2):
                p_tile(t)
            nc.sync.dma_start(
                ag_in_a[:].rearrange("(k2 two p) m -> p k2 two m", p=P, two=2),
                pT_loc[:, :, :, 0:HALF],
            )
            nc.gpsimd.collective_compute(
                "AllGather", ALU.bypass,
                replica_groups=[list(range(NCORES))],
                ins=[ag_in_a[:].opt()], outs=[ag_out_a[:].opt()],
            )
            # ---- p path (second half) ------------------------------------
            for t in range(MT // 2, MT):
                p_tile(t)

            # ---- q path --------------------------------------------------
            for t in range(MT):
                q_tile(t)

            # ---- AllGather B ---------------------------------------------
            nc.sync.dma_start(
                ag_in_b[:].rearrange("(k2 two p) m -> p k2 two m", p=P, two=2),
                pT_loc[:, :, :, HALF:NL],
            )
            nc.gpsimd.collective_compute(
                "AllGather", ALU.bypass,
                replica_groups=[list(range(NCORES))],
                ins=[ag_in_b[:].opt()], outs=[ag_out_b[:].opt()],
            )

            # ---- local warm-up wave (results discarded) ------------------
            for t in range(MT):
                wave_pair(
                    t,
                    lambda k2: pT_loc[:, k2, :, 0:HALF],
                    lambda k2: pT_loc[:, k2, :, HALF:2 * HALF],
                    t * NSLOT + 8,
                )

            # ---- stream gathered blocks into SBUF (gpsimd: idle engine,
            # keeps the long AG waits off the compute streams) -------------
            for r in range(NCORES):
                nc.gpsimd.dma_start(
                    pT_a[:, r * K2:(r + 1) * K2, :, :],
                    ag_out_a[:][r * D:(r + 1) * D, :].rearrange(
                        "(k2 two p) m -> p k2 two m", p=P, two=2
                    ),
                )
            for r in range(NCORES):
                nc.gpsimd.dma_start(
                    pT_b[:, r * K2:(r + 1) * K2, :, :],
                    ag_out_b[:][r * D:(r + 1) * D, :].rearrange(
                        "(k2 two p) m -> p k2 two m", p=P, two=2
                    ),
                )

            # ---- main waves: h outer, rank-pairs inner -------------------
            for h, pT_h in ((0, pT_a), (1, pT_b)):
                for rpair in range(n_waves // 2):
                    r0, r1 = 2 * rpair, 2 * rpair + 1
                    for t in range(MT):
                        wave_pair(
                            t,
                            lambda k2, r0=r0, pT_h=pT_h:
                                pT_h[:, r0 * K2 + k2, :, :],
                            lambda k2, r1=r1, pT_h=pT_h:
                                pT_h[:, r1 * K2 + k2, :, :],
                            t * NSLOT + h * 4 + rpair,
                        )

            # ---- epilogue ------------------------------------------------
            for t in range(MT):
                nc.vector.tensor_reduce(
                    lse_pre[:, t:t + 1],
                    rs[:, t * NSLOT:t * NSLOT + 8],
                    mybir.AxisListType.X,
                    ALU.add,
                )
            nc.scalar.activation(lse[:, :], lse_pre[:, :], AF.Ln)
            nc.vector.tensor_sub(vrow[:, :], lse[:, :], dscaled[:, :])
            nc.vector.tensor_reduce(
                v1[:], vrow[:, :], mybir.AxisListType.X, ALU.add
            )
            pssum = ps.tile([1, 8], f32, tag="ps", name="pssum")
            nc.tensor.matmul(pssum[:], v1[:], ones[:])
            nc.vector.tensor_scalar_mul(ar_sb[:], pssum[:], 1.0 / N)
            nc.sync.dma_start(out_ext.ap(), ar_sb[:, 0:1])

    nc.compile()
    return nc


def _get_nc(inv_temp: float):
    import os
    n_waves = int(os.environ.get("NCE_N_WAVES", NCORES))
    key = (round(float(inv_temp), 9), n_waves)
    if key not in _CACHE:
        _CACHE[key] = _build(inv_temp, n_waves)
    return _CACHE[key]


def kernel(q_emb, p_emb, temperature):
    from concourse.bass_utils import run_bass_kernel_spmd

    q = np.ascontiguousarray(np.asarray(q_emb, dtype=np.float32))
    p = np.ascontiguousarray(np.asarray(p_emb, dtype=np.float32))
    t = float(np.asarray(temperature))
    nc = _get_nc(1.0 / t)

    in_maps = [
        {
            "q": q[i * NL:(i + 1) * NL],
            "p": p[i * NL:(i + 1) * NL],
        }
        for i in range(NCORES)
    ]
    res = run_bass_kernel_spmd(nc, in_maps, core_ids=list(range(NCORES)))
    return np.float32(sum(float(r["out"][0, 0]) for r in res.results))


# revision 29
# speedup vs baseline: 1.7080x; 1.0713x over previous
"""Distributed Trainium2 kernel for in-batch-negative InfoNCE loss.

loss = mean_i( logsumexp_j( cos(q_i, p_j)/T ) - cos(q_i, p_i)/T )

Strategy (8 NeuronCores, data-parallel over N):
  - each core owns a 1024-row shard of q and p (N=8192, D=768)
  - normalize local p shard (x16), transpose into the fp8 DoubleRow
    interleaved layout [d/256, 2, m], AllGather it in two column-chunks
    (the first collective of a NEFF pays a ~60-70us entry barrier;
    chunking lets h=0 wave compute overlap the second chunk)
  - each core computes its 1024 x 8192 slab of logits with fp8e4m3
    DoubleRow matmuls (157 TF/s), fusing exp + row-sum into a 1024-wide
    ScalarE epilogue per psum pair (logits never touch HBM)
  - q is scaled x8 and NOT normalized before the matmul: the
    1/(128*T*||q_i||) row scale is folded into the exp activation
  - diagonal terms computed separately as an elementwise dot of the
    local q/pn shards in f32/bf16 precision
  - a warm-up wave over the local block runs while the AllGather is in
    flight (sums discarded; keeps PE busy and the HAM clock warm)
  - per-core partial sums are returned per core; host adds 8 scalars
"""

import numpy as np

P = 128          # SBUF partitions
D = 768          # embedding dim
K2 = 3           # 256-deep DoubleRow k-tiles
NL = 1024        # local rows per core
MT = NL // P     # 8 m-tiles per core
NCORES = 8
N = NL * NCORES  # 8192
HALF = 512       # column half per rank block
NSLOT = 10       # rs slots per m-tile: 8 real (h*4+rp) + 1 local + pad
QS = 8.0         # q fp8 pre-scale
PS = 16.0        # pn fp8 pre-scale

_CACHE = {}


def _build(inv_temp: float, n_waves: int = NCORES):
    from concourse import bass, bacc, tile, mybir, masks

    f32 = mybir.dt.float32
    bf16 = mybir.dt.bfloat16
    fp8 = mybir.dt.float8e4
    AF = mybir.ActivationFunctionType
    ALU = mybir.AluOpType
    DR = mybir.MatmulPerfMode.DoubleRow

    nc = bacc.Bacc(
        "TRN2",
        debug=False,
        target_bir_lowering=False,
        num_devices=NCORES,
    )

    q_ext = nc.dram_tensor("q", [NL, D], f32, kind="ExternalInput")
    p_ext = nc.dram_tensor("p", [NL, D], f32, kind="ExternalInput")
    out_ext = nc.dram_tensor("out", [1, 1], f32, kind="ExternalOutput")

    with tile.TileContext(nc) as tc:
        with (
            tc.tile_pool(name="singles", bufs=1) as singles,
            tc.tile_pool(name="persist", bufs=1) as persist,
            tc.tile_pool(name="pload", bufs=3) as pload,
            tc.tile_pool(name="qload", bufs=3) as qload,
            tc.tile_pool(name="scr", bufs=4) as scr,
            tc.tile_pool(name="escr", bufs=3) as escr,
            tc.tile_pool(name="stats", bufs=1) as stats,
            tc.tile_pool(name="dram", bufs=1, space="DRAM") as dram,
            tc.tile_pool(name="ps", bufs=2, space="PSUM") as ps,
        ):
            ident_f = singles.tile([P, P], f32)
            masks.make_identity(nc, ident_f[:])
            ident_b = singles.tile([P, P], bf16)
            masks.make_identity(nc, ident_b[:])
            ones = singles.tile([P, 8], f32)
            nc.vector.memset(ones[:], 1.0)
            warm = singles.tile([P, 1], f32)
            nc.vector.memset(warm[:], 0.0)
            bias_p = singles.tile([P, 1], f32)
            nc.vector.memset(bias_p[:], float(np.log(PS)))
            bias_q = singles.tile([P, 1], f32)
            nc.vector.memset(
                bias_q[:], float(np.log(float(inv_temp) / float(QS * PS)))
            )

            # persistent SBUF tensors (fp8 DoubleRow interleaved layouts)
            qT8 = persist.tile([P, K2, 2, NL], fp8)       # 8*q^T
            pT_loc = persist.tile([P, K2, 2, NL], fp8)    # local 16*pn^T
            pn_sb = persist.tile([P, MT, D], bf16)        # local 16*pn
            pT_a = persist.tile([P, NCORES * K2, 2, HALF], fp8)  # cols 0:512
            pT_b = persist.tile([P, NCORES * K2, 2, HALF], fp8)  # cols 512:1024

            # stats
            ssq_p = stats.tile([P, MT], f32)
            ssq_q = stats.tile([P, MT], f32)
            nrm = stats.tile([P, 2 * MT], f32)
            rp = stats.tile([P, MT], f32)        # 16/||p||
            rscale = stats.tile([P, MT], f32)    # 1/(128*T*||q||)
            draw = stats.tile([P, MT], f32)      # raw q.pn16 dots
            dscaled = stats.tile([P, MT], f32)   # diag logits
            rs = stats.tile([P, MT * NSLOT], f32)
            lse_pre = stats.tile([P, MT], f32)
            lse = stats.tile([P, MT], f32)
            vrow = stats.tile([P, MT], f32)
            v1 = stats.tile([P, 1], f32)
            ar_sb = stats.tile([1, 8], f32)

            # DRAM bounce buffers for the collectives
            ag_in_a = dram.tile([D, HALF], fp8)
            ag_in_b = dram.tile([D, HALF], fp8)
            ag_out_a = dram.tile([NCORES * D, HALF], fp8, addr_space="Shared")
            ag_out_b = dram.tile([NCORES * D, HALF], fp8, addr_space="Shared")

            # warm the exp table early
            nc.scalar.activation(warm[:], warm[:], AF.Exp)
            nc.vector.memset(rs[:], 1.0)

            def p_tile(t):
                p_t = pload.tile([P, D], f32, name="p_t")
                nc.sync.dma_start(p_t[:], p_ext.ap()[t * P:(t + 1) * P, :])
                sq = scr.tile([P, D], f32, name="sq")
                nc.vector.scalar_tensor_tensor(
                    out=sq[:], in0=p_t[:], scalar=1.0, in1=p_t[:],
                    op0=ALU.mult, op1=ALU.mult,
                    accum_out=ssq_p[:, t:t + 1],
                )
                # rp = PS/||p|| = exp(-0.5*ln(ssq) + ln(PS)) — stays in the
                # natural_log/exp table set (no Sqrt set swap)
                nc.scalar.activation(nrm[:, t:t + 1], ssq_p[:, t:t + 1], AF.Ln)
                nc.scalar.activation(
                    rp[:, t:t + 1], nrm[:, t:t + 1], AF.Exp,
                    scale=-0.5, bias=bias_p[:],
                )
                nc.vector.tensor_scalar_mul(
                    pn_sb[:, t, :], p_t[:], rp[:, t:t + 1]
                )
                pv = pn_sb[:, t, :].rearrange(
                    "p (k2 c two) -> p k2 two c", k2=K2, two=2
                )
                for k2 in range(K2):
                    for s in range(2):
                        tp = ps.tile([P, P], bf16, tag="ps", name="tp")
                        nc.tensor.transpose(tp[:], pv[:, k2, s, :], ident_b[:])
                        nc.vector.tensor_copy(
                            pT_loc[:, k2, s, t * P:(t + 1) * P], tp[:]
                        )

            def q_tile(t):
                q_t = qload.tile([P, D], f32, name="q_t")
                nc.sync.dma_start(q_t[:], q_ext.ap()[t * P:(t + 1) * P, :])
                sq2 = scr.tile([P, D], f32, name="sq2")
                nc.vector.scalar_tensor_tensor(
                    out=sq2[:], in0=q_t[:], scalar=1.0, in1=q_t[:],
                    op0=ALU.mult, op1=ALU.mult,
                    accum_out=ssq_q[:, t:t + 1],
                )
                # rscale = inv_temp/(QS*PS*||q||) via the same ln/exp set
                nc.scalar.activation(
                    nrm[:, MT + t:MT + t + 1], ssq_q[:, t:t + 1], AF.Ln
                )
                nc.scalar.activation(
                    rscale[:, t:t + 1], nrm[:, MT + t:MT + t + 1], AF.Exp,
                    scale=-0.5, bias=bias_q[:],
                )
                # diag = (q . pn16) * rscale * QS   (rscale has 1/(QS*PS*T))
                dsc = scr.tile([P, D], f32, name="dsc")
                nc.vector.scalar_tensor_tensor(
                    out=dsc[:], in0=q_t[:], scalar=1.0, in1=pn_sb[:, t, :],
                    op0=ALU.mult, op1=ALU.mult,
                    accum_out=draw[:, t:t + 1],
                )
                nc.vector.tensor_mul(
                    dscaled[:, t:t + 1], draw[:, t:t + 1], rscale[:, t:t + 1]
                )
                nc.vector.tensor_scalar_mul(
                    dscaled[:, t:t + 1], dscaled[:, t:t + 1], float(QS)
                )
                qv = q_t[:].rearrange(
                    "p (k2 c two) -> p k2 two c", k2=K2, two=2
                )
                for k2 in range(K2):
                    for s in range(2):
                        tq = ps.tile([P, P], f32, tag="ps", name="tq")
                        nc.tensor.transpose(tq[:], qv[:, k2, s, :], ident_f[:])
                        nc.vector.tensor_scalar_mul(
                            qT8[:, k2, s, t * P:(t + 1) * P], tq[:], float(QS)
                        )

            def wave(t, rhs_fns, slot):
                """len(rhs_fns)*K2 DoubleRow matmuls into a [128, 512*n]
                psum tile (same lhsT reused across the n rhs slices per k2),
                then one fused exp+rowsum. rhs_fns[j]: k2 -> [128,2,512]."""
                n = len(rhs_fns)
                pm = ps.tile([P, n * HALF], f32, tag="ps", name="pm")
                for k2 in range(K2):
                    lhsT = qT8[:, k2, :, t * P:(t + 1) * P]
                    for j, rhs_fn in enumerate(rhs_fns):
                        nc.tensor.matmul(
                            pm[:, j * HALF:(j + 1) * HALF], lhsT, rhs_fn(k2),
                            start=(k2 == 0), stop=(k2 == K2 - 1), perf_mode=DR,
                        )
                ex = escr.tile([P, n * HALF], bf16, tag="ex", name="ex")
                nc.scalar.activation(
                    ex[:], pm[:], AF.Exp,
                    scale=rscale[:, t:t + 1],
                    accum_out=rs[:, slot:slot + 1],
                )

            # ---- p path (first half) -> AllGather A ----------------------
            for t in range(MT // 2):
                p_tile(t)
            nc.sync.dma_start(
                ag_in_a[:].rearrange("(k2 two p) m -> p k2 two m", p=P, two=2),
                pT_loc[:, :, :, 0:HALF],
            )
            nc.gpsimd.collective_compute(
                "AllGather", ALU.bypass,
                replica_groups=[list(range(NCORES))],
                ins=[ag_in_a[:].opt()], outs=[ag_out_a[:].opt()],
            )
            # ---- p path (second half) ------------------------------------
            for t in range(MT // 2, MT):
                p_tile(t)

            # ---- q path --------------------------------------------------
            for t in range(MT):
                q_tile(t)

            # ---- AllGather B ---------------------------------------------
            nc.sync.dma_start(
                ag_in_b[:].rearrange("(k2 two p) m -> p k2 two m", p=P, two=2),
                pT_loc[:, :, :, HALF:NL],
            )
            nc.gpsimd.collective_compute(
                "AllGather", ALU.bypass,
                replica_groups=[list(range(NCORES))],
                ins=[ag_in_b[:].opt()], outs=[ag_out_b[:].opt()],
            )

            # ---- local warm-up wave (results discarded) ------------------
            for t in range(MT):
                wave(
                    t,
                    [lambda k2: pT_loc[:, k2, :, 0:HALF],
                     lambda k2: pT_loc[:, k2, :, HALF:2 * HALF]],
                    t * NSLOT + 8,
                )

            # ---- stream gathered blocks into SBUF (gpsimd: idle engine,
            # keeps the long AG waits off the compute streams) -------------
            for r in range(NCORES):
                nc.gpsimd.dma_start(
                    pT_a[:, r * K2:(r + 1) * K2, :, :],
                    ag_out_a[:][r * D:(r + 1) * D, :].rearrange(
                        "(k2 two p) m -> p k2 two m", p=P, two=2
                    ),
                )
            for r in range(NCORES):
                nc.gpsimd.dma_start(
                    pT_b[:, r * K2:(r + 1) * K2, :, :],
                    ag_out_b[:][r * D:(r + 1) * D, :].rearrange(
                        "(k2 two p) m -> p k2 two m", p=P, two=2
                    ),
                )

            # ---- main waves: h outer, rank-quads inner -------------------
            for h, pT_h in ((0, pT_a), (1, pT_b)):
                for rq in range(n_waves // 4):
                    for t in range(MT):
                        wave(
                            t,
                            [
                                (lambda k2, r=4 * rq + j, pT_h=pT_h:
                                 pT_h[:, r * K2 + k2, :, :])
                                for j in range(4)
                            ],
                            t * NSLOT + h * 2 + rq,
                        )

            # ---- epilogue ------------------------------------------------
            for t in range(MT):
                nc.vector.tensor_reduce(
                    lse_pre[:, t:t + 1],
                    rs[:, t * NSLOT:t * NSLOT + 4],
                    mybir.AxisListType.X,
                    ALU.add,
                )
            nc.scalar.activation(lse[:, :], lse_pre[:, :], AF.Ln)
            nc.vector.tensor_sub(vrow[:, :], lse[:, :], dscaled[:, :])
            nc.vector.tensor_reduce(
                v1[:], vrow[:, :], mybir.AxisListType.X, ALU.add
            )
            pssum = ps.tile([1, 8], f32, tag="ps", name="pssum")
            nc.tensor.matmul(pssum[:], v1[:], ones[:])
            nc.vector.tensor_scalar_mul(ar_sb[:], pssum[:], 1.0 / N)
            nc.sync.dma_start(out_ext.ap(), ar_sb[:, 0:1])

    nc.compile()
    return nc


def _get_nc(inv_temp: float):
    import os
    n_waves = int(os.environ.get("NCE_N_WAVES", NCORES))
    key = (round(float(inv_temp), 9), n_waves)
    if key not in _CACHE:
        _CACHE[key] = _build(inv_temp, n_waves)
    return _CACHE[key]


def kernel(q_emb, p_emb, temperature):
    from concourse.bass_utils import run_bass_kernel_spmd

    q = np.ascontiguousarray(np.asarray(q_emb, dtype=np.float32))
    p = np.ascontiguousarray(np.asarray(p_emb, dtype=np.float32))
    t = float(np.asarray(temperature))
    nc = _get_nc(1.0 / t)

    in_maps = [
        {
            "q": q[i * NL:(i + 1) * NL],
            "p": p[i * NL:(i + 1) * NL],
        }
        for i in range(NCORES)
    ]
    res = run_bass_kernel_spmd(nc, in_maps, core_ids=list(range(NCORES)))
    return np.float32(sum(float(r["out"][0, 0]) for r in res.results))
